# revision 28
# baseline (speedup 1.0000x reference)
"""MoE layer (top-2 of 8 experts) on 8 Trainium2 NeuronCores.

Strategy (expert-parallel, matching the sharding hint):
  - Host computes the gate (logits -> top-2 -> softmax) and the aux loss;
    this is 0.05% of the FLOPs.
  - Tokens are dispatched per expert on the host (the "all-to-all"), padded
    to a fixed capacity C, and core e runs expert e's FFN:
        y = gate * (GELU(x @ W1[e] + b1[e]) @ W2[e])
    on its block of routed tokens.  Both GEMMs run in bf16 with fp32 PSUM
    accumulation.  The first GEMM is computed transposed (A^T = W1^T @ x^T)
    so the second GEMM needs no on-device transpose.
  - Host scatter-adds the per-expert results back into token order and adds
    the (tiny) gate-weighted b2 term.

Device time is dominated by the two GEMMs: 2 * C * (D*H + H*DOUT) flops
per core ~ 19 GFLOP, near the bf16 PE roofline.
"""

import numpy as np
import ml_dtypes

# ---- problem constants (hardcoded per contract) ----
B, S, D, DOUT, E, H, K = 2, 2048, 1024, 1024, 8, 4096, 2
N = B * S
AUX_COEFF = 0.01
P = 128
C = 1088            # per-expert token capacity per round (max count is 1066)
C_PAD = 1152        # C rounded up to a multiple of 128 (DRAM layout granularity)
KD = D // P         # 8  contraction chunks, GEMM1
MH = H // P         # 32 output row tiles, GEMM1 (H on partitions)
KH = H // P         # 32 contraction chunks, GEMM2
CT = C_PAD // P     # 9  output row tiles, GEMM2 (tokens on partitions)
C_TILES = [(0, 512), (512, 512), (1024, C - 1024)]   # GEMM1 free-dim tiling
M_TILES = [(i * P, min(P, C - i * P)) for i in range(CT) if i * P < C]

BF16 = ml_dtypes.bfloat16

_COMPILED = None  # cached (nc, run) so repeated kernel() calls reuse the NEFF
LAST_EXEC_NS = None   # max-core HW exec time from the last traced run
LAST_TRACE = None     # path to the last perfetto trace (trace runs only)


def _build_program(act="gelu"):
    import concourse.mybir as mybir
    import concourse.tile as tile
    from concourse import bacc

    act_fn = {
        "gelu": mybir.ActivationFunctionType.Gelu,
        # CoreSim has no Gelu LUT; identity lets the sim validate everything else
        "identity": mybir.ActivationFunctionType.Identity,
    }[act]

    f32 = mybir.dt.float32
    bf16 = mybir.dt.bfloat16

    nc = bacc.Bacc(
        "TRN2",
        target_bir_lowering=False,
        debug=False,
        enable_asserts=True,
        num_devices=8,
    )

    # Inputs are pre-laid-out on the host so every DMA is contiguous:
    #   xt : [P, KD, C]  bf16   xt[p,k,c]  = x_routed[c, k*P+p]
    #   w1 : [MH, P, KD, P] bf16 w1[m,p,k,j] = W1[k*P+p, m*P+j]
    #   w2 : [P, KH, DOUT] bf16 w2[p,k,n]  = W2[k*P+p, n]
    #   b1 : [P, MH] f32        b1[p,m]    = b1[m*P+p]
    #   gv : [P, CT] f32        gv[p,t]    = gate[t*P+p]
    xt_d = nc.dram_tensor("xt", [P, KD, C], bf16, kind="ExternalInput").ap()
    w1_d = nc.dram_tensor("w1", [MH, P, KD, P], bf16, kind="ExternalInput").ap()
    w2_d = nc.dram_tensor("w2", [P, KH, DOUT], bf16, kind="ExternalInput").ap()
    b1_d = nc.dram_tensor("b1v", [P, MH], f32, kind="ExternalInput").ap()
    g_d = nc.dram_tensor("gv", [P, CT], f32, kind="ExternalInput").ap()
    y_d = nc.dram_tensor("y", [CT, P, DOUT], f32, kind="ExternalOutput").ap()

    with tile.TileContext(nc) as tc:
        with (
            tc.tile_pool(name="resident", bufs=1) as res,
            tc.tile_pool(name="w1s", bufs=3) as w1s,
            tc.tile_pool(name="yout", bufs=4) as yout,
            tc.tile_pool(name="ps", bufs=8, space="PSUM") as psp,
        ):
            # The first matmul needs only xt chunk 0 + w1 tile 0 — issue those
            # first so compute starts ~10us in instead of waiting for the full
            # activation load.
            xt_sb = res.tile([P, KD, C], bf16)
            nc.sync.dma_start(xt_sb[:, 0], xt_d[:, 0])
            w1_t0 = w1s.tile([P, KD, P], bf16, tag="w1t", name="w1_t0")
            nc.sync.dma_start(w1_t0[:], w1_d[0])
            for k in range(1, KD):
                nc.sync.dma_start(xt_sb[:, k], xt_d[:, k])
            b1_sb = res.tile([P, MH], f32)
            nc.sync.dma_start(b1_sb[:], b1_d[:])
            g_sb = res.tile([P, CT], f32)
            nc.sync.dma_start(g_sb[:], g_d[:])
            w2_sb = res.tile([P, KH, DOUT], bf16)
            hmt_sb = res.tile([P, KH, C], bf16)

            # PE warmup: the HAM clock gate holds the PE at 1.2 GHz until it
            # has been busy ~3.4us.  Burn that window on scratch matmuls while
            # the input DMAs stream, so real matmuls start at 2.4 GHz.
            warm_sb = res.tile([P, P], bf16)
            nc.any.memset(warm_sb[:], 0.0)
            warm_ps = psp.tile([P, P], f32, tag="ps", name="warm_ps")
            for i in range(40):
                nc.tensor.matmul(
                    warm_ps[:], lhsT=warm_sb[:], rhs=warm_sb[:],
                    start=(i == 0), stop=(i == 39),
                )

            # GEMM1: hmt[:, m, :] = GELU(W1_m^T @ x^T + b1_m), H on partitions
            for m in range(MH):
                if m == 0:
                    w1_t = w1_t0
                else:
                    w1_t = w1s.tile([P, KD, P], bf16, tag="w1t")
                    nc.sync.dma_start(w1_t[:], w1_d[m])
                # stream one W2 chunk per GEMM1 m-tile: keeps the startup
                # DMA path short while W2 fully lands before GEMM2 begins
                nc.sync.dma_start(w2_sb[:, m], w2_d[:, m])
                ps_tiles = []
                for noff, nsz in C_TILES:
                    ps_tiles.append(psp.tile([P, nsz], f32, tag="ps", name=f"ps1_{m}_{noff}"))
                for k in range(KD):
                    for (noff, nsz), ps in zip(C_TILES, ps_tiles):
                        nc.tensor.matmul(
                            ps[:],
                            lhsT=w1_t[:, k],
                            rhs=xt_sb[:, k, noff : noff + nsz],
                            start=(k == 0),
                            stop=(k == KD - 1),
                        )
                for (noff, nsz), ps in zip(C_TILES, ps_tiles):
                    nc.scalar.activation(
                        hmt_sb[:, m, noff : noff + nsz],
                        ps[:],
                        act_fn,
                        bias=b1_sb[:, m : m + 1],
                        scale=1.0,
                    )

            # GEMM2: y[mc] = gate * (Hm_mc @ W2), tokens on partitions
            for mc, (roff, rows) in enumerate(M_TILES):
                ps0 = psp.tile([P, 512], f32, tag="ps", name=f"ps2a_{mc}")
                ps1 = psp.tile([P, 512], f32, tag="ps", name=f"ps2b_{mc}")
                for k in range(KH):
                    lhs = hmt_sb[:, k, roff : roff + rows]
                    nc.tensor.matmul(
                        ps0[:rows], lhsT=lhs, rhs=w2_sb[:, k, 0:512],
                        start=(k == 0), stop=(k == KH - 1),
                    )
                    nc.tensor.matmul(
                        ps1[:rows], lhsT=lhs, rhs=w2_sb[:, k, 512:1024],
                        start=(k == 0), stop=(k == KH - 1),
                    )
                # b2 is added on the host (comb_w @ b2); device applies only
                # the gate scale, split across DVE and ACT so both halves
                # run in parallel.
                y_t = yout.tile([P, DOUT], f32, tag="yt")
                nc.vector.tensor_scalar_mul(
                    y_t[:rows, 0:512], ps0[:rows], g_sb[:rows, mc : mc + 1]
                )
                nc.scalar.mul(y_t[:rows, 512:1024], ps1[:rows], g_sb[:rows, mc : mc + 1])
                nc.sync.dma_start(y_d[mc, :rows], y_t[:rows])

    nc.compile()
    return nc


def _get_program():
    global _COMPILED
    if _COMPILED is None:
        _COMPILED = _build_program()
    return _COMPILED


def _route(x_flat, Wg, bg):
    """Host gating: logits, top-2 (matches jax.lax.top_k tie-breaking),
    softmax gates, aux loss."""
    logits = x_flat.astype(np.float32) @ Wg.astype(np.float32) + bg.astype(np.float32)
    order = np.argsort(-logits, axis=-1, kind="stable")[:, :K]       # [N, K]
    topv = np.take_along_axis(logits, order, axis=1)                 # [N, K]
    mx = topv.max(axis=1, keepdims=True)
    eg = np.exp(topv - mx)
    gates = eg / eg.sum(axis=1, keepdims=True)                       # [N, K]

    lmx = logits.max(axis=1, keepdims=True)
    lse = np.log(np.exp(logits - lmx).sum(axis=1, keepdims=True)) + lmx
    log_probs = logits - lse
    ideal = 1.0 / E
    aux = AUX_COEFF * np.mean(ideal * (np.log(ideal) - log_probs), dtype=np.float64)
    return order, gates, np.float32(aux)


def kernel(x, Wg, bg, W1, b1, W2, b2):
    from concourse.bass_utils import run_bass_kernel_spmd

    x = np.asarray(x)
    x_flat = np.ascontiguousarray(x.reshape(N, D), dtype=np.float32)
    Wg, bg = np.asarray(Wg), np.asarray(bg)
    W1, b1, W2, b2 = (np.asarray(a, dtype=np.float32) for a in (W1, b1, W2, b2))

    order, gates, aux = _route(x_flat, Wg, bg)

    # per-expert token index lists and combined gate weights
    idx_e, g_e = [], []
    for e in range(E):
        hits = order == e                           # [N, K]
        w = (gates * hits).sum(axis=1).astype(np.float32)
        idx = np.nonzero(hits.any(axis=1))[0]
        idx_e.append(idx)
        g_e.append(w[idx])
    max_cnt = max(len(i) for i in idx_e)
    rounds = max(1, -(-max_cnt // C))

    # static per-core tensors (weights), laid out for contiguous DMA
    x_bf = x_flat.astype(BF16)
    static_maps = []
    for e in range(E):
        w1h = np.ascontiguousarray(
            W1[e].astype(BF16).reshape(KD, P, MH, P).transpose(2, 1, 0, 3)
        )                                            # [MH, P, KD, P]
        w2h = np.ascontiguousarray(
            W2[e].astype(BF16).reshape(KH, P, DOUT).transpose(1, 0, 2)
        )                                            # [P, KH, DOUT]
        b1h = np.ascontiguousarray(b1[e].reshape(MH, P).T)           # [P, MH]
        static_maps.append({"w1": w1h, "w2": w2h, "b1v": b1h})

    nc = _get_program()

    # b2 contribution, applied on the host: out[n] += sum_k gate[n,k]*b2[e_nk]
    out_flat = gates[:, 0, None] * b2[order[:, 0]] + gates[:, 1, None] * b2[order[:, 1]]
    out_flat = out_flat.astype(np.float32)
    for r in range(rounds):
        in_maps = []
        chunk_idx = []
        for e in range(E):
            idx = idx_e[e][r * C : (r + 1) * C]
            g = g_e[e][r * C : (r + 1) * C]
            chunk_idx.append(idx)
            xg = np.zeros((C, D), dtype=BF16)
            xg[: len(idx)] = x_bf[idx]
            xth = np.ascontiguousarray(
                xg.reshape(C, KD, P).transpose(2, 1, 0)
            )                                        # [P, KD, C]
            gp = np.zeros(C_PAD, dtype=np.float32)
            gp[: len(g)] = g
            gh = np.ascontiguousarray(gp.reshape(CT, P).T)           # [P, CT]
            in_maps.append({**static_maps[e], "xt": xth, "gv": gh})

        import os

        global LAST_EXEC_NS, LAST_TRACE
        trace = bool(int(os.environ.get("KERNEL_TRACE", "0")))
        res = run_bass_kernel_spmd(nc, in_maps, list(range(E)), trace=trace)
        if trace:
            LAST_EXEC_NS = res.exec_time_ns
            if res.instructions_and_trace is not None:
                LAST_TRACE = res.instructions_and_trace[1]
        results = res.results

        for e in range(E):
            idx = chunk_idx[e]
            y = np.asarray(results[e]["y"]).reshape(C_PAD, DOUT)
            # idx is unique within an expert, so fancy-index += is safe
            out_flat[idx] += y[: len(idx)]

    return out_flat.reshape(B, S, DOUT), aux


# revision 38
# speedup vs baseline: 1.0275x; 1.0275x over previous
"""MoE layer (top-2 of 8 experts) on 8 Trainium2 NeuronCores.

Strategy (expert-parallel, matching the sharding hint):
  - Host computes the gate (logits -> top-2 -> softmax) and the aux loss;
    this is 0.05% of the FLOPs.
  - Tokens are dispatched per expert on the host (the "all-to-all"), padded
    to a fixed capacity C, and core e runs expert e's FFN:
        y = gate * (GELU(x @ W1[e] + b1[e]) @ W2[e])
    on its block of routed tokens.  Both GEMMs run in bf16 with fp32 PSUM
    accumulation.  The first GEMM is computed transposed (A^T = W1^T @ x^T)
    so the second GEMM needs no on-device transpose.
  - Host scatter-adds the per-expert results back into token order and adds
    the (tiny) gate-weighted b2 term.

Device time is dominated by the two GEMMs: 2 * C * (D*H + H*DOUT) flops
per core ~ 19 GFLOP, near the bf16 PE roofline.
"""

import numpy as np
import ml_dtypes

# ---- problem constants (hardcoded per contract) ----
B, S, D, DOUT, E, H, K = 2, 2048, 1024, 1024, 8, 4096, 2
N = B * S
AUX_COEFF = 0.01
P = 128
C = 1088            # per-expert token capacity per round (max count is 1066)
C_PAD = 1152        # C rounded up to a multiple of 128 (DRAM layout granularity)
KD = D // P         # 8  contraction chunks, GEMM1
MH = H // P         # 32 output row tiles, GEMM1 (H on partitions)
KH = H // P         # 32 contraction chunks, GEMM2
MD = DOUT // P      # 8  output row tiles, GEMM2 (transposed: DOUT on partitions)
C_TILES = [(0, 512), (512, 512), (1024, C - 1024)]   # token free-dim tiling

BF16 = ml_dtypes.bfloat16

_COMPILED = None  # cached (nc, run) so repeated kernel() calls reuse the NEFF
LAST_EXEC_NS = None   # max-core HW exec time from the last traced run
LAST_TRACE = None     # path to the last perfetto trace (trace runs only)


def _build_program(act="gelu"):
    import concourse.mybir as mybir
    import concourse.tile as tile
    from concourse import bacc

    act_fn = {
        "gelu": mybir.ActivationFunctionType.Gelu,
        # CoreSim has no Gelu LUT; identity lets the sim validate everything else
        "identity": mybir.ActivationFunctionType.Identity,
    }[act]

    f32 = mybir.dt.float32
    bf16 = mybir.dt.bfloat16

    nc = bacc.Bacc(
        "TRN2",
        target_bir_lowering=False,
        debug=False,
        enable_asserts=True,
        num_devices=8,
    )

    # Inputs are pre-laid-out on the host so every DMA is contiguous:
    #   xt : [P, KD, C]  bf16   xt[p,k,c]  = x_routed[c, k*P+p]
    #   w1 : [MH, P, KD, P] bf16 w1[m,p,k,j] = W1[k*P+p, m*P+j]
    #   w2 : [P, KH, DOUT] bf16 w2[p,k,n]  = W2[k*P+p, n]
    #   b1 : [P, MH] f32        b1[p,m]    = b1[m*P+p]
    #   gv : [P, C] f32         per-token gate, pre-broadcast over partitions
    # Output y is the TRANSPOSED result: y[m,p,c] = out[c, m*P+p], so GEMM2
    # puts DOUT on partitions (exactly MD=8 tiles — no token-padding tile).
    xt_d = nc.dram_tensor("xt", [P, KD, C], bf16, kind="ExternalInput").ap()
    w1_d = nc.dram_tensor("w1", [MH, P, KD, P], bf16, kind="ExternalInput").ap()
    w2_d = nc.dram_tensor("w2", [P, KH, DOUT], bf16, kind="ExternalInput").ap()
    b1_d = nc.dram_tensor("b1v", [P, MH], f32, kind="ExternalInput").ap()
    g_d = nc.dram_tensor("gv", [P, C], f32, kind="ExternalInput").ap()
    y_d = nc.dram_tensor("y", [MD, P, C], f32, kind="ExternalOutput").ap()

    with tile.TileContext(nc) as tc:
        with (
            tc.tile_pool(name="resident", bufs=1) as res,
            tc.tile_pool(name="w1s", bufs=3) as w1s,
            tc.tile_pool(name="yout", bufs=4) as yout,
            tc.tile_pool(name="ps", bufs=8, space="PSUM") as psp,
        ):
            # The first matmul needs only xt chunk 0 + w1 tile 0 — issue those
            # first so compute starts ~10us in instead of waiting for the full
            # activation load.
            xt_sb = res.tile([P, KD, C], bf16)
            nc.sync.dma_start(xt_sb[:, 0], xt_d[:, 0])
            w1_t0 = w1s.tile([P, KD, P], bf16, tag="w1t", name="w1_t0")
            nc.sync.dma_start(w1_t0[:], w1_d[0])
            for k in range(1, KD):
                nc.sync.dma_start(xt_sb[:, k], xt_d[:, k])
            b1_sb = res.tile([P, MH], f32)
            nc.sync.dma_start(b1_sb[:], b1_d[:])
            g_sb = res.tile([P, C], f32)
            nc.sync.dma_start(g_sb[:], g_d[:])
            w2_sb = res.tile([P, KH, DOUT], bf16)
            hmt_sb = res.tile([P, KH, C], bf16)

            # PE warmup: the HAM clock gate holds the PE at 1.2 GHz until it
            # has been busy ~3.4us.  Burn that window on scratch matmuls while
            # the input DMAs stream, so real matmuls start at 2.4 GHz.
            warm_sb = res.tile([P, P], bf16)
            nc.any.memset(warm_sb[:], 0.0)
            warm_ps = psp.tile([P, P], f32, tag="ps", name="warm_ps")
            for i in range(40):
                nc.tensor.matmul(
                    warm_ps[:], lhsT=warm_sb[:], rhs=warm_sb[:],
                    start=(i == 0), stop=(i == 39),
                )

            # GEMM1: hmt[:, m, :] = GELU(W1_m^T @ x^T + b1_m), H on partitions
            for m in range(MH):
                if m == 0:
                    w1_t = w1_t0
                else:
                    w1_t = w1s.tile([P, KD, P], bf16, tag="w1t")
                    nc.sync.dma_start(w1_t[:], w1_d[m])
                # stream one W2 chunk per GEMM1 m-tile: keeps the startup
                # DMA path short while W2 fully lands before GEMM2 begins
                nc.sync.dma_start(w2_sb[:, m], w2_d[:, m])
                ps_tiles = []
                for noff, nsz in C_TILES:
                    ps_tiles.append(psp.tile([P, nsz], f32, tag="ps", name=f"ps1_{m}_{noff}"))
                for k in range(KD):
                    for (noff, nsz), ps in zip(C_TILES, ps_tiles):
                        nc.tensor.matmul(
                            ps[:],
                            lhsT=w1_t[:, k],
                            rhs=xt_sb[:, k, noff : noff + nsz],
                            start=(k == 0),
                            stop=(k == KD - 1),
                        )
                for (noff, nsz), ps in zip(C_TILES, ps_tiles):
                    nc.scalar.activation(
                        hmt_sb[:, m, noff : noff + nsz],
                        ps[:],
                        act_fn,
                        bias=b1_sb[:, m : m + 1],
                        scale=1.0,
                    )

            # GEMM2 (transposed): y^T[m] = gate ⊙ (W2_m^T @ Hm^T), DOUT on
            # partitions so the streamed dim is C (no token-padding tile).
            # b2 is added on the host (gate-weighted gather).
            for m in range(MD):
                ps_tiles = []
                for noff, nsz in C_TILES:
                    ps_tiles.append(psp.tile([P, nsz], f32, tag="ps", name=f"ps2_{m}_{noff}"))
                for k in range(KH):
                    lhs = w2_sb[:, k, m * P : (m + 1) * P]
                    for (noff, nsz), ps in zip(C_TILES, ps_tiles):
                        nc.tensor.matmul(
                            ps[:], lhsT=lhs, rhs=hmt_sb[:, k, noff : noff + nsz],
                            start=(k == 0), stop=(k == KH - 1),
                        )
                y_t = yout.tile([P, C], f32, tag="yt")
                for (noff, nsz), ps in zip(C_TILES, ps_tiles):
                    nc.vector.tensor_mul(
                        y_t[:, noff : noff + nsz], ps[:], g_sb[:, noff : noff + nsz]
                    )
                nc.sync.dma_start(y_d[m], y_t[:])

    nc.compile()
    return nc


def _get_program():
    global _COMPILED
    if _COMPILED is None:
        _COMPILED = _build_program()
    return _COMPILED


def _route(x_flat, Wg, bg):
    """Host gating: logits, top-2 (matches jax.lax.top_k tie-breaking),
    softmax gates, aux loss."""
    logits = x_flat.astype(np.float32) @ Wg.astype(np.float32) + bg.astype(np.float32)
    order = np.argsort(-logits, axis=-1, kind="stable")[:, :K]       # [N, K]
    topv = np.take_along_axis(logits, order, axis=1)                 # [N, K]
    mx = topv.max(axis=1, keepdims=True)
    eg = np.exp(topv - mx)
    gates = eg / eg.sum(axis=1, keepdims=True)                       # [N, K]

    lmx = logits.max(axis=1, keepdims=True)
    lse = np.log(np.exp(logits - lmx).sum(axis=1, keepdims=True)) + lmx
    log_probs = logits - lse
    ideal = 1.0 / E
    aux = AUX_COEFF * np.mean(ideal * (np.log(ideal) - log_probs), dtype=np.float64)
    return order, gates, np.float32(aux)


def kernel(x, Wg, bg, W1, b1, W2, b2):
    from concourse.bass_utils import run_bass_kernel_spmd

    x = np.asarray(x)
    x_flat = np.ascontiguousarray(x.reshape(N, D), dtype=np.float32)
    Wg, bg = np.asarray(Wg), np.asarray(bg)
    W1, b1, W2, b2 = (np.asarray(a, dtype=np.float32) for a in (W1, b1, W2, b2))

    order, gates, aux = _route(x_flat, Wg, bg)

    # per-expert token index lists and combined gate weights
    idx_e, g_e = [], []
    for e in range(E):
        hits = order == e                           # [N, K]
        w = (gates * hits).sum(axis=1).astype(np.float32)
        idx = np.nonzero(hits.any(axis=1))[0]
        idx_e.append(idx)
        g_e.append(w[idx])
    max_cnt = max(len(i) for i in idx_e)
    rounds = max(1, -(-max_cnt // C))

    # static per-core tensors (weights), laid out for contiguous DMA
    x_bf = x_flat.astype(BF16)
    static_maps = []
    for e in range(E):
        w1h = np.ascontiguousarray(
            W1[e].astype(BF16).reshape(KD, P, MH, P).transpose(2, 1, 0, 3)
        )                                            # [MH, P, KD, P]
        w2h = np.ascontiguousarray(
            W2[e].astype(BF16).reshape(KH, P, DOUT).transpose(1, 0, 2)
        )                                            # [P, KH, DOUT]
        b1h = np.ascontiguousarray(b1[e].reshape(MH, P).T)           # [P, MH]
        static_maps.append({"w1": w1h, "w2": w2h, "b1v": b1h})

    nc = _get_program()

    # b2 contribution, applied on the host: out[n] += sum_k gate[n,k]*b2[e_nk]
    out_flat = gates[:, 0, None] * b2[order[:, 0]] + gates[:, 1, None] * b2[order[:, 1]]
    out_flat = out_flat.astype(np.float32)
    for r in range(rounds):
        in_maps = []
        chunk_idx = []
        for e in range(E):
            idx = idx_e[e][r * C : (r + 1) * C]
            g = g_e[e][r * C : (r + 1) * C]
            chunk_idx.append(idx)
            xg = np.zeros((C, D), dtype=BF16)
            xg[: len(idx)] = x_bf[idx]
            xth = np.ascontiguousarray(
                xg.reshape(C, KD, P).transpose(2, 1, 0)
            )                                        # [P, KD, C]
            gp = np.zeros(C, dtype=np.float32)
            gp[: len(g)] = g
            gh = np.ascontiguousarray(np.broadcast_to(gp, (P, C)))   # [P, C]
            in_maps.append({**static_maps[e], "xt": xth, "gv": gh})

        import os

        global LAST_EXEC_NS, LAST_TRACE
        trace = bool(int(os.environ.get("KERNEL_TRACE", "0")))
        res = run_bass_kernel_spmd(nc, in_maps, list(range(E)), trace=trace)
        if trace:
            LAST_EXEC_NS = res.exec_time_ns
            if res.instructions_and_trace is not None:
                LAST_TRACE = res.instructions_and_trace[1]
        results = res.results

        for e in range(E):
            idx = chunk_idx[e]
            # y is transposed on device: y[m, p, c] = out[c, m*P+p]
            yt = np.asarray(results[e]["y"]).reshape(DOUT, C)
            # idx is unique within an expert, so fancy-index += is safe
            out_flat[idx] += yt[:, : len(idx)].T

    return out_flat.reshape(B, S, DOUT), aux


# revision 42
# speedup vs baseline: 1.0349x; 1.0071x over previous
"""MoE layer (top-2 of 8 experts) on 8 Trainium2 NeuronCores.

Strategy (expert-parallel, matching the sharding hint):
  - Host computes the gate (logits -> top-2 -> softmax) and the aux loss;
    this is 0.05% of the FLOPs.
  - Tokens are dispatched per expert on the host (the "all-to-all"), padded
    to a fixed capacity C, and core e runs expert e's FFN:
        y = gate * (GELU(x @ W1[e] + b1[e]) @ W2[e])
    on its block of routed tokens.  Both GEMMs run in bf16 with fp32 PSUM
    accumulation.  The first GEMM is computed transposed (A^T = W1^T @ x^T)
    so the second GEMM needs no on-device transpose.
  - Host scatter-adds the per-expert results back into token order and adds
    the (tiny) gate-weighted b2 term.

Device time is dominated by the two GEMMs: 2 * C * (D*H + H*DOUT) flops
per core ~ 19 GFLOP, near the bf16 PE roofline.
"""

import numpy as np
import ml_dtypes

# ---- problem constants (hardcoded per contract) ----
B, S, D, DOUT, E, H, K = 2, 2048, 1024, 1024, 8, 4096, 2
N = B * S
AUX_COEFF = 0.01
P = 128
C = 1088            # per-expert token capacity per round (max count is 1066)
C_PAD = 1152        # C rounded up to a multiple of 128 (DRAM layout granularity)
KD = D // P         # 8  contraction chunks, GEMM1
MH = H // P         # 32 output row tiles, GEMM1 (H on partitions)
KH = H // P         # 32 contraction chunks, GEMM2
MD = DOUT // P      # 8  output row tiles, GEMM2 (transposed: DOUT on partitions)
C_TILES = [(0, 512), (512, 512), (1024, C - 1024)]   # token free-dim tiling

BF16 = ml_dtypes.bfloat16

_COMPILED = None  # cached (nc, run) so repeated kernel() calls reuse the NEFF
LAST_EXEC_NS = None   # max-core HW exec time from the last traced run
LAST_TRACE = None     # path to the last perfetto trace (trace runs only)


def _build_program(act="gelu"):
    import concourse.mybir as mybir
    import concourse.tile as tile
    from concourse import bacc

    act_fn = {
        "gelu": mybir.ActivationFunctionType.Gelu,
        # CoreSim has no Gelu LUT; identity lets the sim validate everything else
        "identity": mybir.ActivationFunctionType.Identity,
    }[act]

    f32 = mybir.dt.float32
    bf16 = mybir.dt.bfloat16

    nc = bacc.Bacc(
        "TRN2",
        target_bir_lowering=False,
        debug=False,
        enable_asserts=True,
        num_devices=8,
    )

    # Inputs are pre-laid-out on the host so every DMA is contiguous:
    #   xt : [P, KD, C]  bf16   xt[p,k,c]  = x_routed[c, k*P+p]
    #   w1 : [MH, P, KD, P] bf16 w1[m,p,k,j] = W1[k*P+p, m*P+j]
    #   w2 : [P, KH, DOUT] bf16 w2[p,k,n]  = W2[k*P+p, n]
    #   b1 : [P, MH] f32        b1[p,m]    = b1[m*P+p]
    #   gv : [P, C] f32         per-token gate, pre-broadcast over partitions
    # Output y is the TRANSPOSED result: y[m,p,c] = out[c, m*P+p], so GEMM2
    # puts DOUT on partitions (exactly MD=8 tiles — no token-padding tile).
    xt_d = nc.dram_tensor("xt", [P, KD, C], bf16, kind="ExternalInput").ap()
    w1_d = nc.dram_tensor("w1", [MH, P, KD, P], bf16, kind="ExternalInput").ap()
    w2_d = nc.dram_tensor("w2", [P, KH, DOUT], bf16, kind="ExternalInput").ap()
    b1_d = nc.dram_tensor("b1v", [P, MH], f32, kind="ExternalInput").ap()
    g_d = nc.dram_tensor("gv", [P, C], f32, kind="ExternalInput").ap()
    y_d = nc.dram_tensor("y", [MD, P, C], f32, kind="ExternalOutput").ap()

    with tile.TileContext(nc) as tc:
        with (
            tc.tile_pool(name="resident", bufs=1) as res,
            tc.tile_pool(name="w1s", bufs=3) as w1s,
            tc.tile_pool(name="yout", bufs=4) as yout,
            tc.tile_pool(name="ps", bufs=8, space="PSUM") as psp,
        ):
            # The first matmul needs only xt chunk 0 + w1 tile 0 — issue those
            # first so compute starts ~10us in instead of waiting for the full
            # activation load.
            xt_sb = res.tile([P, KD, C], bf16)
            nc.sync.dma_start(xt_sb[:, 0], xt_d[:, 0])
            w1_t0 = w1s.tile([P, KD, P], bf16, tag="w1t", name="w1_t0")
            nc.sync.dma_start(w1_t0[:], w1_d[0])
            for k in range(1, KD):
                nc.sync.dma_start(xt_sb[:, k], xt_d[:, k])
            b1_sb = res.tile([P, MH], f32)
            nc.sync.dma_start(b1_sb[:], b1_d[:])
            g_sb = res.tile([P, C], f32)
            w2_sb = res.tile([P, KH, DOUT], bf16)
            hmt_sb = res.tile([P, KH, C], bf16)

            # PE warmup: the HAM clock gate holds the PE at 1.2 GHz until it
            # has been busy ~3.4us.  Burn that window on scratch matmuls while
            # the input DMAs stream, so real matmuls start at 2.4 GHz.
            warm_sb = res.tile([P, P], bf16)
            nc.any.memset(warm_sb[:], 0.0)
            warm_ps = psp.tile([P, P], f32, tag="ps", name="warm_ps")
            for i in range(40):
                nc.tensor.matmul(
                    warm_ps[:], lhsT=warm_sb[:], rhs=warm_sb[:],
                    start=(i == 0), stop=(i == 39),
                )

            # GEMM1: hmt[:, m, :] = GELU(W1_m^T @ x^T + b1_m), H on partitions
            for m in range(MH):
                if m == 0:
                    w1_t = w1_t0
                else:
                    w1_t = w1s.tile([P, KD, P], bf16, tag="w1t")
                    nc.sync.dma_start(w1_t[:], w1_d[m])
                # stream one W2 chunk per GEMM1 m-tile (offset by 4 so the
                # first w1 prefetches aren't delayed); the remainder plus the
                # gate tile follow after the loop, well before GEMM2 needs them
                if m >= 4:
                    nc.sync.dma_start(w2_sb[:, m - 4], w2_d[:, m - 4])
                ps_tiles = []
                for noff, nsz in C_TILES:
                    ps_tiles.append(psp.tile([P, nsz], f32, tag="ps", name=f"ps1_{m}_{noff}"))
                for k in range(KD):
                    for (noff, nsz), ps in zip(C_TILES, ps_tiles):
                        nc.tensor.matmul(
                            ps[:],
                            lhsT=w1_t[:, k],
                            rhs=xt_sb[:, k, noff : noff + nsz],
                            start=(k == 0),
                            stop=(k == KD - 1),
                        )
                for (noff, nsz), ps in zip(C_TILES, ps_tiles):
                    nc.scalar.activation(
                        hmt_sb[:, m, noff : noff + nsz],
                        ps[:],
                        act_fn,
                        bias=b1_sb[:, m : m + 1],
                        scale=1.0,
                    )

            for kk in range(MH - 4, MH):
                nc.sync.dma_start(w2_sb[:, kk], w2_d[:, kk])
            nc.sync.dma_start(g_sb[:], g_d[:])

            # GEMM2 (transposed): y^T[m] = gate ⊙ (W2_m^T @ Hm^T), DOUT on
            # partitions so the streamed dim is C (no token-padding tile).
            # b2 is added on the host (gate-weighted gather).
            for m in range(MD):
                ps_tiles = []
                for noff, nsz in C_TILES:
                    ps_tiles.append(psp.tile([P, nsz], f32, tag="ps", name=f"ps2_{m}_{noff}"))
                for k in range(KH):
                    lhs = w2_sb[:, k, m * P : (m + 1) * P]
                    for (noff, nsz), ps in zip(C_TILES, ps_tiles):
                        nc.tensor.matmul(
                            ps[:], lhsT=lhs, rhs=hmt_sb[:, k, noff : noff + nsz],
                            start=(k == 0), stop=(k == KH - 1),
                        )
                y_t = yout.tile([P, C], f32, tag="yt")
                for (noff, nsz), ps in zip(C_TILES, ps_tiles):
                    nc.vector.tensor_mul(
                        y_t[:, noff : noff + nsz], ps[:], g_sb[:, noff : noff + nsz]
                    )
                    # per-chunk store so the transfer overlaps the remaining
                    # epilogue (shortens the kernel tail)
                    nc.sync.dma_start(
                        y_d[m, :, noff : noff + nsz], y_t[:, noff : noff + nsz]
                    )

    nc.compile()
    return nc


def _get_program():
    global _COMPILED
    if _COMPILED is None:
        _COMPILED = _build_program()
    return _COMPILED


def _route(x_flat, Wg, bg):
    """Host gating: logits, top-2 (matches jax.lax.top_k tie-breaking),
    softmax gates, aux loss."""
    logits = x_flat.astype(np.float32) @ Wg.astype(np.float32) + bg.astype(np.float32)
    order = np.argsort(-logits, axis=-1, kind="stable")[:, :K]       # [N, K]
    topv = np.take_along_axis(logits, order, axis=1)                 # [N, K]
    mx = topv.max(axis=1, keepdims=True)
    eg = np.exp(topv - mx)
    gates = eg / eg.sum(axis=1, keepdims=True)                       # [N, K]

    lmx = logits.max(axis=1, keepdims=True)
    lse = np.log(np.exp(logits - lmx).sum(axis=1, keepdims=True)) + lmx
    log_probs = logits - lse
    ideal = 1.0 / E
    aux = AUX_COEFF * np.mean(ideal * (np.log(ideal) - log_probs), dtype=np.float64)
    return order, gates, np.float32(aux)


def kernel(x, Wg, bg, W1, b1, W2, b2):
    from concourse.bass_utils import run_bass_kernel_spmd

    x = np.asarray(x)
    x_flat = np.ascontiguousarray(x.reshape(N, D), dtype=np.float32)
    Wg, bg = np.asarray(Wg), np.asarray(bg)
    W1, b1, W2, b2 = (np.asarray(a, dtype=np.float32) for a in (W1, b1, W2, b2))

    order, gates, aux = _route(x_flat, Wg, bg)

    # per-expert token index lists and combined gate weights
    idx_e, g_e = [], []
    for e in range(E):
        hits = order == e                           # [N, K]
        w = (gates * hits).sum(axis=1).astype(np.float32)
        idx = np.nonzero(hits.any(axis=1))[0]
        idx_e.append(idx)
        g_e.append(w[idx])
    max_cnt = max(len(i) for i in idx_e)
    rounds = max(1, -(-max_cnt // C))

    # static per-core tensors (weights), laid out for contiguous DMA
    x_bf = x_flat.astype(BF16)
    static_maps = []
    for e in range(E):
        w1h = np.ascontiguousarray(
            W1[e].astype(BF16).reshape(KD, P, MH, P).transpose(2, 1, 0, 3)
        )                                            # [MH, P, KD, P]
        w2h = np.ascontiguousarray(
            W2[e].astype(BF16).reshape(KH, P, DOUT).transpose(1, 0, 2)
        )                                            # [P, KH, DOUT]
        b1h = np.ascontiguousarray(b1[e].reshape(MH, P).T)           # [P, MH]
        static_maps.append({"w1": w1h, "w2": w2h, "b1v": b1h})

    nc = _get_program()

    # b2 contribution, applied on the host: out[n] += sum_k gate[n,k]*b2[e_nk]
    out_flat = gates[:, 0, None] * b2[order[:, 0]] + gates[:, 1, None] * b2[order[:, 1]]
    out_flat = out_flat.astype(np.float32)
    for r in range(rounds):
        in_maps = []
        chunk_idx = []
        for e in range(E):
            idx = idx_e[e][r * C : (r + 1) * C]
            g = g_e[e][r * C : (r + 1) * C]
            chunk_idx.append(idx)
            xg = np.zeros((C, D), dtype=BF16)
            xg[: len(idx)] = x_bf[idx]
            xth = np.ascontiguousarray(
                xg.reshape(C, KD, P).transpose(2, 1, 0)
            )                                        # [P, KD, C]
            gp = np.zeros(C, dtype=np.float32)
            gp[: len(g)] = g
            gh = np.ascontiguousarray(np.broadcast_to(gp, (P, C)))   # [P, C]
            in_maps.append({**static_maps[e], "xt": xth, "gv": gh})

        import os

        global LAST_EXEC_NS, LAST_TRACE
        trace = bool(int(os.environ.get("KERNEL_TRACE", "0")))
        res = run_bass_kernel_spmd(nc, in_maps, list(range(E)), trace=trace)
        if trace:
            LAST_EXEC_NS = res.exec_time_ns
            if res.instructions_and_trace is not None:
                LAST_TRACE = res.instructions_and_trace[1]
        results = res.results

        for e in range(E):
            idx = chunk_idx[e]
            # y is transposed on device: y[m, p, c] = out[c, m*P+p]
            yt = np.asarray(results[e]["y"]).reshape(DOUT, C)
            # idx is unique within an expert, so fancy-index += is safe
            out_flat[idx] += yt[:, : len(idx)].T

    return out_flat.reshape(B, S, DOUT), aux


# revision 45
# speedup vs baseline: 1.0475x; 1.0122x over previous
"""MoE layer (top-2 of 8 experts) on 8 Trainium2 NeuronCores.

Strategy (expert-parallel, matching the sharding hint):
  - Host computes the gate (logits -> top-2 -> softmax) and the aux loss;
    this is 0.05% of the FLOPs.
  - Tokens are dispatched per expert on the host (the "all-to-all"), padded
    to a fixed capacity C, and core e runs expert e's FFN:
        y = gate * (GELU(x @ W1[e] + b1[e]) @ W2[e])
    on its block of routed tokens.  Both GEMMs run in bf16 with fp32 PSUM
    accumulation.  The first GEMM is computed transposed (A^T = W1^T @ x^T)
    so the second GEMM needs no on-device transpose.
  - Host scatter-adds the per-expert results back into token order and adds
    the (tiny) gate-weighted b2 term.

Device time is dominated by the two GEMMs: 2 * C * (D*H + H*DOUT) flops
per core ~ 19 GFLOP, near the bf16 PE roofline.
"""

import numpy as np
import ml_dtypes

# ---- problem constants (hardcoded per contract) ----
B, S, D, DOUT, E, H, K = 2, 2048, 1024, 1024, 8, 4096, 2
N = B * S
AUX_COEFF = 0.01
P = 128
C = 1088            # per-expert token capacity per round (max count is 1066)
C_PAD = 1152        # C rounded up to a multiple of 128 (DRAM layout granularity)
KD = D // P         # 8  contraction chunks, GEMM1
MH = H // P         # 32 output row tiles, GEMM1 (H on partitions)
KH = H // P         # 32 contraction chunks, GEMM2
MD = DOUT // P      # 8  output row tiles, GEMM2 (transposed: DOUT on partitions)
C_TILES = [(0, 512), (512, 512), (1024, C - 1024)]   # token free-dim tiling

BF16 = ml_dtypes.bfloat16

_COMPILED = None  # cached (nc, run) so repeated kernel() calls reuse the NEFF
LAST_EXEC_NS = None   # max-core HW exec time from the last traced run
LAST_TRACE = None     # path to the last perfetto trace (trace runs only)


def _build_program(act="gelu"):
    import concourse.mybir as mybir
    import concourse.tile as tile
    from concourse import bacc

    act_fn = {
        "gelu": mybir.ActivationFunctionType.Gelu,
        # CoreSim has no Gelu LUT; identity lets the sim validate everything else
        "identity": mybir.ActivationFunctionType.Identity,
    }[act]

    f32 = mybir.dt.float32
    bf16 = mybir.dt.bfloat16

    nc = bacc.Bacc(
        "TRN2",
        target_bir_lowering=False,
        debug=False,
        enable_asserts=True,
        num_devices=8,
    )

    # Inputs are pre-laid-out on the host so every DMA is contiguous:
    #   xt : [P, KD, C]  bf16   xt[p,k,c]  = x_routed[c, k*P+p]
    #   w1 : [MH, P, KD, P] bf16 w1[m,p,k,j] = W1[k*P+p, m*P+j]
    #   w2 : [P, KH, DOUT] bf16 w2[p,k,n]  = W2[k*P+p, n]
    #   b1 : [P, MH] f32        b1[p,m]    = b1[m*P+p]
    #   gv : [P, C] f32         per-token gate, pre-broadcast over partitions
    # Output y is the TRANSPOSED result: y[m,p,c] = out[c, m*P+p], so GEMM2
    # puts DOUT on partitions (exactly MD=8 tiles — no token-padding tile).
    xt_d = nc.dram_tensor("xt", [P, KD, C], bf16, kind="ExternalInput").ap()
    w1_d = nc.dram_tensor("w1", [MH, P, KD, P], bf16, kind="ExternalInput").ap()
    w2_d = nc.dram_tensor("w2", [P, KH, DOUT], bf16, kind="ExternalInput").ap()
    b1_d = nc.dram_tensor("b1v", [P, MH], f32, kind="ExternalInput").ap()
    g_d = nc.dram_tensor("gv", [P, C], f32, kind="ExternalInput").ap()
    y_d = nc.dram_tensor("y", [MD, P, C], f32, kind="ExternalOutput").ap()

    with tile.TileContext(nc) as tc:
        with (
            tc.tile_pool(name="resident", bufs=1) as res,
            tc.tile_pool(name="w1s", bufs=6) as w1s,
            tc.tile_pool(name="yout", bufs=4) as yout,
            tc.tile_pool(name="ps", bufs=8, space="PSUM") as psp,
        ):
            # The first matmul needs only xt chunk 0 + w1 tile 0.  Interleave
            # the first four w1 tiles among the early xt chunks so m=1..3
            # never wait behind the full 2.2MB activation stream.
            xt_sb = res.tile([P, KD, C], bf16)
            w1_pre = {}

            def _prefetch_w1(m):
                t = w1s.tile([P, KD, P], bf16, tag="w1t", name=f"w1_pre{m}")
                nc.sync.dma_start(t[:], w1_d[m])
                w1_pre[m] = t

            nc.sync.dma_start(xt_sb[:, 0], xt_d[:, 0])
            _prefetch_w1(0)
            _prefetch_w1(1)
            nc.sync.dma_start(xt_sb[:, 1], xt_d[:, 1])
            nc.sync.dma_start(xt_sb[:, 2], xt_d[:, 2])
            _prefetch_w1(2)
            nc.sync.dma_start(xt_sb[:, 3], xt_d[:, 3])
            _prefetch_w1(3)
            for k in range(4, KD):
                nc.sync.dma_start(xt_sb[:, k], xt_d[:, k])
            b1_sb = res.tile([P, MH], f32)
            nc.sync.dma_start(b1_sb[:], b1_d[:])
            g_sb = res.tile([P, C], f32)
            w2_sb = res.tile([P, KH, DOUT], bf16)
            hmt_sb = res.tile([P, KH, C], bf16)

            # PE warmup: the HAM clock gate holds the PE at 1.2 GHz until it
            # has been busy ~3.4us.  Burn that window on scratch matmuls while
            # the input DMAs stream, so real matmuls start at 2.4 GHz.
            warm_sb = res.tile([P, P], bf16)
            nc.any.memset(warm_sb[:], 0.0)
            warm_ps = psp.tile([P, P], f32, tag="ps", name="warm_ps")
            for i in range(40):
                nc.tensor.matmul(
                    warm_ps[:], lhsT=warm_sb[:], rhs=warm_sb[:],
                    start=(i == 0), stop=(i == 39),
                )

            # GEMM1: hmt[:, m, :] = GELU(W1_m^T @ x^T + b1_m), H on partitions
            for m in range(MH):
                if m in w1_pre:
                    w1_t = w1_pre.pop(m)
                else:
                    w1_t = w1s.tile([P, KD, P], bf16, tag="w1t")
                    nc.sync.dma_start(w1_t[:], w1_d[m])
                # stream one W2 chunk per GEMM1 m-tile (offset by 4 so the
                # first w1 prefetches aren't delayed); the remainder plus the
                # gate tile follow after the loop, well before GEMM2 needs them
                if m >= 4:
                    nc.sync.dma_start(w2_sb[:, m - 4], w2_d[:, m - 4])
                ps_tiles = []
                for noff, nsz in C_TILES:
                    ps_tiles.append(psp.tile([P, nsz], f32, tag="ps", name=f"ps1_{m}_{noff}"))
                for k in range(KD):
                    for (noff, nsz), ps in zip(C_TILES, ps_tiles):
                        nc.tensor.matmul(
                            ps[:],
                            lhsT=w1_t[:, k],
                            rhs=xt_sb[:, k, noff : noff + nsz],
                            start=(k == 0),
                            stop=(k == KD - 1),
                        )
                for (noff, nsz), ps in zip(C_TILES, ps_tiles):
                    nc.scalar.activation(
                        hmt_sb[:, m, noff : noff + nsz],
                        ps[:],
                        act_fn,
                        bias=b1_sb[:, m : m + 1],
                        scale=1.0,
                    )

            for kk in range(MH - 4, MH):
                nc.sync.dma_start(w2_sb[:, kk], w2_d[:, kk])
            nc.sync.dma_start(g_sb[:], g_d[:])

            # GEMM2 (transposed): y^T[m] = gate ⊙ (W2_m^T @ Hm^T), DOUT on
            # partitions so the streamed dim is C (no token-padding tile).
            # b2 is added on the host (gate-weighted gather).
            for m in range(MD):
                ps_tiles = []
                for noff, nsz in C_TILES:
                    ps_tiles.append(psp.tile([P, nsz], f32, tag="ps", name=f"ps2_{m}_{noff}"))
                for k in range(KH):
                    lhs = w2_sb[:, k, m * P : (m + 1) * P]
                    for (noff, nsz), ps in zip(C_TILES, ps_tiles):
                        nc.tensor.matmul(
                            ps[:], lhsT=lhs, rhs=hmt_sb[:, k, noff : noff + nsz],
                            start=(k == 0), stop=(k == KH - 1),
                        )
                y_t = yout.tile([P, C], f32, tag="yt")
                for (noff, nsz), ps in zip(C_TILES, ps_tiles):
                    nc.vector.tensor_mul(
                        y_t[:, noff : noff + nsz], ps[:], g_sb[:, noff : noff + nsz]
                    )
                    # per-chunk store so the transfer overlaps the remaining
                    # epilogue (shortens the kernel tail)
                    nc.sync.dma_start(
                        y_d[m, :, noff : noff + nsz], y_t[:, noff : noff + nsz]
                    )

    nc.compile()
    return nc


def _get_program():
    global _COMPILED
    if _COMPILED is None:
        _COMPILED = _build_program()
    return _COMPILED


def _route(x_flat, Wg, bg):
    """Host gating: logits, top-2 (matches jax.lax.top_k tie-breaking),
    softmax gates, aux loss."""
    logits = x_flat.astype(np.float32) @ Wg.astype(np.float32) + bg.astype(np.float32)
    order = np.argsort(-logits, axis=-1, kind="stable")[:, :K]       # [N, K]
    topv = np.take_along_axis(logits, order, axis=1)                 # [N, K]
    mx = topv.max(axis=1, keepdims=True)
    eg = np.exp(topv - mx)
    gates = eg / eg.sum(axis=1, keepdims=True)                       # [N, K]

    lmx = logits.max(axis=1, keepdims=True)
    lse = np.log(np.exp(logits - lmx).sum(axis=1, keepdims=True)) + lmx
    log_probs = logits - lse
    ideal = 1.0 / E
    aux = AUX_COEFF * np.mean(ideal * (np.log(ideal) - log_probs), dtype=np.float64)
    return order, gates, np.float32(aux)


def kernel(x, Wg, bg, W1, b1, W2, b2):
    from concourse.bass_utils import run_bass_kernel_spmd

    x = np.asarray(x)
    x_flat = np.ascontiguousarray(x.reshape(N, D), dtype=np.float32)
    Wg, bg = np.asarray(Wg), np.asarray(bg)
    W1, b1, W2, b2 = (np.asarray(a, dtype=np.float32) for a in (W1, b1, W2, b2))

    order, gates, aux = _route(x_flat, Wg, bg)

    # per-expert token index lists and combined gate weights
    idx_e, g_e = [], []
    for e in range(E):
        hits = order == e                           # [N, K]
        w = (gates * hits).sum(axis=1).astype(np.float32)
        idx = np.nonzero(hits.any(axis=1))[0]
        idx_e.append(idx)
        g_e.append(w[idx])
    max_cnt = max(len(i) for i in idx_e)
    rounds = max(1, -(-max_cnt // C))

    # static per-core tensors (weights), laid out for contiguous DMA
    x_bf = x_flat.astype(BF16)
    static_maps = []
    for e in range(E):
        w1h = np.ascontiguousarray(
            W1[e].astype(BF16).reshape(KD, P, MH, P).transpose(2, 1, 0, 3)
        )                                            # [MH, P, KD, P]
        w2h = np.ascontiguousarray(
            W2[e].astype(BF16).reshape(KH, P, DOUT).transpose(1, 0, 2)
        )                                            # [P, KH, DOUT]
        b1h = np.ascontiguousarray(b1[e].reshape(MH, P).T)           # [P, MH]
        static_maps.append({"w1": w1h, "w2": w2h, "b1v": b1h})

    nc = _get_program()

    # b2 contribution, applied on the host: out[n] += sum_k gate[n,k]*b2[e_nk]
    out_flat = gates[:, 0, None] * b2[order[:, 0]] + gates[:, 1, None] * b2[order[:, 1]]
    out_flat = out_flat.astype(np.float32)
    for r in range(rounds):
        in_maps = []
        chunk_idx = []
        for e in range(E):
            idx = idx_e[e][r * C : (r + 1) * C]
            g = g_e[e][r * C : (r + 1) * C]
            chunk_idx.append(idx)
            xg = np.zeros((C, D), dtype=BF16)
            xg[: len(idx)] = x_bf[idx]
            xth = np.ascontiguousarray(
                xg.reshape(C, KD, P).transpose(2, 1, 0)
            )                                        # [P, KD, C]
            gp = np.zeros(C, dtype=np.float32)
            gp[: len(g)] = g
            gh = np.ascontiguousarray(np.broadcast_to(gp, (P, C)))   # [P, C]
            in_maps.append({**static_maps[e], "xt": xth, "gv": gh})

        import os

        global LAST_EXEC_NS, LAST_TRACE
        trace = bool(int(os.environ.get("KERNEL_TRACE", "0")))
        res = run_bass_kernel_spmd(nc, in_maps, list(range(E)), trace=trace)
        if trace:
            LAST_EXEC_NS = res.exec_time_ns
            if res.instructions_and_trace is not None:
                LAST_TRACE = res.instructions_and_trace[1]
        results = res.results

        for e in range(E):
            idx = chunk_idx[e]
            # y is transposed on device: y[m, p, c] = out[c, m*P+p]
            yt = np.asarray(results[e]["y"]).reshape(DOUT, C)
            # idx is unique within an expert, so fancy-index += is safe
            out_flat[idx] += yt[:, : len(idx)].T

    return out_flat.reshape(B, S, DOUT), aux
